# revision 1
# baseline (speedup 1.0000x reference)
"""Classical Hopfield one-sweep asynchronous update on Trainium2 (Bass).

Structure exploited: the Hebbian weights satisfy W + I = U U^T exactly with
rank R=128 (U recovered by host-side pivoted Cholesky in fp64).  One full
asynchronous sweep in `perm` order then reduces to 64 blocks of 128 neurons:

  m = U^T s0                                  (host, 128-vector)
  per block b:  v = Ub @ m - s0p_b + eps      (PE)
                C = (-2 s0p_b * Ub) @ Ub^T    (PE, block interaction rows)
                128-step serial sign chain    (DVE: gate + fused AXPY per step)
                m += Ug_b^T g                 (PE)

All per-block operands stream from DRAM; C rows are repacked to partition 0
via an SBUF->SBUF DMA so the serial chain runs entirely on one engine with
static access patterns.  An eps=1e-3 bias makes device signs provably equal
to the fp32 jax reference (activations are exact multiples of 1/128; all
device errors are < 1e-4).  The gate vector G is returned and applied to the
state on the host.  All 8 cores run the identical program (the serial chain
cannot be sharded); core 0's output is used.

This toolchain's walrus accepts only ONE semaphore wait per instruction, so a
post-scheduling pass hoists extra waits into EventSemaphore carriers.
"""

from contextlib import ExitStack

import numpy as np

import concourse.bass as bass
import concourse.mybir as mybir
from concourse import tile
from concourse.bass_utils import run_bass_kernel_spmd

F32 = mybir.dt.float32
EPS = 1e-3
N, R, B = 8192, 128, 128
NB = N // B
S = 4 * B


def _split_multi_waits(nc, max_waits=1):
    n = 0
    for fn in nc.m.functions:
        for blk in fn.blocks:
            insts = blk.instructions
            i = 0
            while i < len(insts):
                inst = insts[i]
                si = inst.sync_info
                if si is not None and len(si.on_wait) > max_waits:
                    waits = list(si.on_wait)
                    keep, extra = waits[-max_waits:], waits[:-max_waits]
                    for j, w in enumerate(extra):
                        ev = mybir.InstEventSemaphore(name=f"waitfix_{n}")
                        n += 1
                        ev.engine = inst.engine
                        ev.sync_info = mybir.SyncInfo(on_wait=[w], on_update=[])
                        insts.insert(i + j, ev)
                    inst.sync_info = mybir.SyncInfo(
                        on_wait=keep, on_update=list(si.on_update)
                    )
                    i += len(extra) + 1
                else:
                    i += 1
    return n


def _build_nc():
    nc = bass.Bass("TRN2", target_bir_lowering=False, debug=False)

    blk = nc.dram_tensor("blk", [128, NB * S], F32, kind="ExternalInput")
    ns0p = nc.dram_tensor("ns0p", [1, N], F32, kind="ExternalInput")
    m0 = nc.dram_tensor("m0", [R, 1], F32, kind="ExternalInput")
    gout = nc.dram_tensor("gout", [1, N], F32, kind="ExternalOutput")

    mult = mybir.AluOpType.mult
    add = mybir.AluOpType.add
    is_gt = mybir.AluOpType.is_gt

    with tile.TileContext(nc) as tc, ExitStack() as ctx:
        slices = ctx.enter_context(tc.tile_pool(name="slices", bufs=4))
        strips = ctx.enter_context(tc.tile_pool(name="strips", bufs=2))
        csb = ctx.enter_context(tc.tile_pool(name="csb", bufs=4))
        cps = ctx.enter_context(tc.tile_pool(name="cps", bufs=2, space="PSUM"))
        vps = ctx.enter_context(tc.tile_pool(name="vps", bufs=2, space="PSUM"))
        bps = ctx.enter_context(tc.tile_pool(name="bps", bufs=2, space="PSUM"))
        chain = ctx.enter_context(tc.tile_pool(name="chain", bufs=4))
        persist = ctx.enter_context(tc.tile_pool(name="persist", bufs=1))

        m_sb = persist.tile([R, 1], F32)
        one_sb = persist.tile([1, 1], F32)
        ns0p_sb = persist.tile([1, N], F32)
        nc.sync.dma_start(m_sb[:], m0[:, :])
        nc.sync.dma_start(ns0p_sb[:], ns0p[:, :])
        nc.vector.memset(one_sb[:], 1.0)

        def load_blk(b):
            blk_sl = slices.tile([128, S], F32, tag="blk_sl")
            nc.sync.dma_start(blk_sl[:], blk[:, b * S:(b + 1) * S])
            return blk_sl

        def build_strip(blk_sl):
            upt_sl = blk_sl[:R, 0:B]
            uptg_sl = blk_sl[:R, B:2 * B]
            c_ps = cps.tile([B, B], F32, tag="c_ps")
            nc.tensor.matmul(c_ps[:], uptg_sl, upt_sl, start=True, stop=True)
            c_sb = csb.tile([B, B], F32, tag="c_sb")
            nc.scalar.copy(c_sb[:], c_ps[:])
            strip = strips.tile([1, B * B], F32, tag="strip")
            nc.sync.dma_start(
                strip[0:1, :].rearrange("o (k j) -> o k j", k=B, j=B), c_sb[:]
            )
            return strip

        def v_matmuls(blk_sl):
            v_ps = vps.tile([1, B], F32, tag="v_ps")
            nc.tensor.matmul(v_ps[:], m_sb[:], blk_sl[:R, 0:B], start=True, stop=False)
            nc.tensor.matmul(
                v_ps[:], one_sb[:], blk_sl[0:1, 3 * B:4 * B], start=False, stop=True
            )
            return v_ps

        def init_w(v_ps):
            w = chain.tile([1, B], F32, tag="w")
            nc.vector.tensor_scalar(w[:], v_ps[:], EPS, None, add)
            return w

        cur = load_blk(0)
        cur_strip = build_strip(cur)
        w = init_w(v_matmuls(cur))

        for b in range(NB):
            blk_sl = cur
            strip = cur_strip
            if b + 1 < NB:
                cur = load_blk(b + 1)
                cur_strip = build_strip(cur)

            ns0p_row = ns0p_sb[0:1, b * B:(b + 1) * B]
            grow = chain.tile([1, B], F32, tag="grow")
            for k in range(B):
                nc.vector.tensor_scalar(
                    grow[0:1, k:k + 1], w[0:1, k:k + 1],
                    ns0p_row[0:1, k:k + 1], 0.0, mult, is_gt,
                )
                if k + 1 < B:
                    nc.vector.scalar_tensor_tensor(
                        w[0:1, k + 1:B],
                        strip[0:1, k * B + k + 1:k * B + B],
                        grow[0:1, k:k + 1],
                        w[0:1, k + 1:B],
                        mult, add,
                    )

            nc.sync.dma_start(gout[:, b * B:(b + 1) * B], grow[:])

            if b + 1 < NB:
                gcol_ps = bps.tile([B, 1], F32, tag="gcol_ps")
                nc.tensor.matmul(gcol_ps[:], grow[:], one_sb[:], start=True, stop=True)
                gcol_sb = chain.tile([B, 1], F32, tag="gcol_sb")
                nc.vector.tensor_copy(gcol_sb[:], gcol_ps[:])
                dm_ps = bps.tile([R, 1], F32, tag="dm_ps")
                nc.tensor.matmul(
                    dm_ps[:], blk_sl[:B, 2 * B:2 * B + R], gcol_sb[:],
                    start=True, stop=True,
                )
                nc.vector.tensor_tensor(m_sb[:], m_sb[:], dm_ps[:], add)
                w = init_w(v_matmuls(cur))

    _split_multi_waits(nc)
    return nc


_NC_CACHE = None


def _get_nc():
    global _NC_CACHE
    if _NC_CACHE is None:
        _NC_CACHE = _build_nc()
    return _NC_CACHE


def _factor_U(W):
    """Pivoted Cholesky of W+I in fp64; returns U [N,R] fp32 or None."""
    A = W.astype(np.float64) + np.eye(N)
    diag = np.diagonal(A).copy()
    L = np.zeros((N, R))
    for r in range(R):
        j = int(np.argmax(diag))
        if diag[j] < 1e-10:
            L = L[:, :r]
            break
        ljj = np.sqrt(diag[j])
        L[:, r] = (A[:, j] - L[:, :r] @ L[j, :r]) / ljj
        diag -= L[:, r] ** 2
        diag[j] = 0.0
        np.maximum(diag, 0, out=diag)
    U = np.zeros((N, R))
    U[:, :L.shape[1]] = L
    # spot-check the factorization
    idx = np.linspace(0, N - 1, 64).astype(np.int64)
    res = np.abs(U[idx] @ U.T - A[idx]).max()
    return (U.astype(np.float32), float(res))


def _pack_inputs(U, s0, perm):
    Up = U[perm].astype(np.float32)
    s0p = s0[perm].astype(np.float32)
    Ug = (-2.0 * s0p[:, None] * Up).astype(np.float32)
    blk = np.zeros((128, NB * S), dtype=np.float32)
    for b in range(NB):
        sl = slice(b * B, (b + 1) * B)
        blk[:R, b * S + 0:b * S + B] = Up[sl].T
        blk[:R, b * S + B:b * S + 2 * B] = Ug[sl].T
        blk[:B, b * S + 2 * B:b * S + 2 * B + R] = Ug[sl]
        blk[0, b * S + 3 * B:b * S + 4 * B] = -s0p[sl]
    m0 = (U.T.astype(np.float32) @ s0.astype(np.float32))[:, None].astype(np.float32)
    return {"blk": blk, "ns0p": (-s0p)[None, :].astype(np.float32), "m0": m0}


def _sweep_numpy(W, s, perm):
    """Exact fp32 sequential fallback (used only if W is not Hebbian rank-128)."""
    s = s.astype(np.float32).copy()
    for i in perm:
        act = np.float32(np.dot(W[i].astype(np.float32), s))
        s[i] = np.float32(1.0) if act >= 0 else np.float32(-1.0)
    return s


def kernel(W, state, perm, num_iterations):
    W = np.asarray(W, dtype=np.float32)
    state = np.asarray(state, dtype=np.float32)
    perm_i = np.asarray(perm).astype(np.int64)
    n_it = int(np.asarray(num_iterations))

    s = state.copy()
    if n_it <= 0:
        return s

    U, res = _factor_U(W)
    if res > 1e-4:
        for _ in range(n_it):
            s = _sweep_numpy(W, s, perm_i)
        return s

    nc = _get_nc()
    core_ids = list(range(8))
    for _ in range(n_it):
        ins = _pack_inputs(U, s, perm_i)
        r = run_bass_kernel_spmd(nc, [dict(ins) for _ in core_ids], core_ids)
        G = r.results[0]["gout"].reshape(-1)
        flip = perm_i[G > 0.5]
        s[flip] = -s[flip]
    return s


# revision 2
# speedup vs baseline: 1.0154x; 1.0154x over previous
"""Classical Hopfield one-sweep asynchronous update on Trainium2 (Bass).

Structure exploited: the Hebbian weights satisfy W + I = U U^T exactly with
rank R=128 (U recovered by host-side pivoted Cholesky in fp64).  One full
asynchronous sweep in `perm` order then reduces to 64 blocks of 128 neurons:

  m = U^T s0                                  (host, 128-vector)
  per block b:  v = Ub @ m - s0p_b + eps      (PE)
                C = (-2 s0p_b * Ub) @ Ub^T    (PE, block interaction rows)
                128-step serial sign chain    (DVE: gate + fused AXPY per step)
                m += Ug_b^T g                 (PE)

All per-block operands stream from DRAM; C rows are repacked to partition 0
via an SBUF->SBUF DMA so the serial chain runs entirely on one engine with
static access patterns.  An eps=1e-3 bias makes device signs provably equal
to the fp32 jax reference (activations are exact multiples of 1/128; all
device errors are < 1e-4).  The gate vector G is returned and applied to the
state on the host.  All 8 cores run the identical program (the serial chain
cannot be sharded); core 0's output is used.

This toolchain's walrus accepts only ONE semaphore wait per instruction, so a
post-scheduling pass hoists extra waits into EventSemaphore carriers.
"""

from contextlib import ExitStack

import numpy as np

import concourse.bass as bass
import concourse.mybir as mybir
from concourse import tile
from concourse.bass_utils import run_bass_kernel_spmd

F32 = mybir.dt.float32
EPS = 1e-3
N, R, B = 8192, 128, 128
NB = N // B
S = 4 * B


def _split_multi_waits(nc, max_waits=1):
    n = 0
    for fn in nc.m.functions:
        for blk in fn.blocks:
            insts = blk.instructions
            i = 0
            while i < len(insts):
                inst = insts[i]
                si = inst.sync_info
                if si is not None and len(si.on_wait) > max_waits:
                    waits = list(si.on_wait)
                    keep, extra = waits[-max_waits:], waits[:-max_waits]
                    for j, w in enumerate(extra):
                        ev = mybir.InstEventSemaphore(name=f"waitfix_{n}")
                        n += 1
                        ev.engine = inst.engine
                        ev.sync_info = mybir.SyncInfo(on_wait=[w], on_update=[])
                        insts.insert(i + j, ev)
                    inst.sync_info = mybir.SyncInfo(
                        on_wait=keep, on_update=list(si.on_update)
                    )
                    i += len(extra) + 1
                else:
                    i += 1
    return n


def _build_nc():
    nc = bass.Bass("TRN2", target_bir_lowering=False, debug=False)

    blk = nc.dram_tensor("blk", [128, NB * S], F32, kind="ExternalInput")
    ns0p = nc.dram_tensor("ns0p", [1, N], F32, kind="ExternalInput")
    m0 = nc.dram_tensor("m0", [R, 1], F32, kind="ExternalInput")
    gout = nc.dram_tensor("gout", [1, N], F32, kind="ExternalOutput")

    mult = mybir.AluOpType.mult
    add = mybir.AluOpType.add
    is_gt = mybir.AluOpType.is_gt

    with tile.TileContext(nc) as tc, ExitStack() as ctx:
        slices = ctx.enter_context(tc.tile_pool(name="slices", bufs=4))
        strips = ctx.enter_context(tc.tile_pool(name="strips", bufs=2))
        csb = ctx.enter_context(tc.tile_pool(name="csb", bufs=4))
        cps = ctx.enter_context(tc.tile_pool(name="cps", bufs=2, space="PSUM"))
        vps = ctx.enter_context(tc.tile_pool(name="vps", bufs=2, space="PSUM"))
        bps = ctx.enter_context(tc.tile_pool(name="bps", bufs=2, space="PSUM"))
        eps_p = ctx.enter_context(tc.tile_pool(name="eps_p", bufs=2, space="PSUM"))
        esb_p = ctx.enter_context(tc.tile_pool(name="esb_p", bufs=2))
        chain = ctx.enter_context(tc.tile_pool(name="chain", bufs=4))
        persist = ctx.enter_context(tc.tile_pool(name="persist", bufs=1))

        m_sb = persist.tile([R, 1], F32)
        one_sb = persist.tile([1, 1], F32)
        ns0p_sb = persist.tile([1, N], F32)
        nc.sync.dma_start(m_sb[:], m0[:, :])
        nc.sync.dma_start(ns0p_sb[:], ns0p[:, :])
        nc.vector.memset(one_sb[:], 1.0)

        def load_blk(b):
            blk_sl = slices.tile([128, S], F32, tag="blk_sl")
            nc.sync.dma_start(blk_sl[:], blk[:, b * S:(b + 1) * S])
            return blk_sl

        def build_strip(blk_sl):
            upt_sl = blk_sl[:R, 0:B]
            uptg_sl = blk_sl[:R, B:2 * B]
            c_ps = cps.tile([B, B], F32, tag="c_ps")
            nc.tensor.matmul(c_ps[:], uptg_sl, upt_sl, start=True, stop=True)
            c_sb = csb.tile([B, B], F32, tag="c_sb")
            nc.scalar.copy(c_sb[:], c_ps[:])
            strip = strips.tile([1, B * B], F32, tag="strip")
            nc.sync.dma_start(
                strip[0:1, :].rearrange("o (k j) -> o k j", k=B, j=B), c_sb[:]
            )
            return strip

        def v_matmuls(blk_sl, close=True):
            v_ps = vps.tile([1, B], F32, tag="v_ps")
            nc.tensor.matmul(v_ps[:], m_sb[:], blk_sl[:R, 0:B], start=True, stop=False)
            nc.tensor.matmul(
                v_ps[:], one_sb[:], blk_sl[0:1, 3 * B:4 * B], start=False, stop=close
            )
            return v_ps

        def build_e(blk_b, blk_b1):
            # E[k, j] = Ug_b[k] . Up_{b+1}[j]  (v correction at the boundary)
            e_ps = eps_p.tile([B, B], F32, tag="e_ps")
            nc.tensor.matmul(e_ps[:], blk_b[:R, B:2 * B], blk_b1[:R, 0:B],
                             start=True, stop=True)
            e_sb = esb_p.tile([B, B], F32, tag="e_sb")
            nc.scalar.copy(e_sb[:], e_ps[:])
            return e_sb

        def init_w(v_ps):
            w = chain.tile([1, B], F32, tag="w")
            nc.vector.tensor_scalar(w[:], v_ps[:], EPS, None, add)
            return w

        cur = load_blk(0)
        cur_strip = build_strip(cur)
        w = init_w(v_matmuls(cur, close=True))

        for b in range(NB):
            blk_sl = cur
            strip = cur_strip
            v_next = None
            if b + 1 < NB:
                cur = load_blk(b + 1)
                cur_strip = build_strip(cur)
                e_sb = build_e(blk_sl, cur)
                v_next = v_matmuls(cur, close=False)

            ns0p_row = ns0p_sb[0:1, b * B:(b + 1) * B]
            grow = chain.tile([1, B], F32, tag="grow")
            for k in range(B):
                nc.vector.tensor_scalar(
                    grow[0:1, k:k + 1], w[0:1, k:k + 1],
                    ns0p_row[0:1, k:k + 1], 0.0, mult, is_gt,
                )
                if k + 1 < B:
                    nc.vector.scalar_tensor_tensor(
                        w[0:1, k + 1:B],
                        strip[0:1, k * B + k + 1:k * B + B],
                        grow[0:1, k:k + 1],
                        w[0:1, k + 1:B],
                        mult, add,
                    )

            nc.sync.dma_start(gout[:, b * B:(b + 1) * B], grow[:])

            if b + 1 < NB:
                # critical path: G -> Gcol -> v_next += Gcol^T E ; m update off-path
                gcol_ps = bps.tile([B, 1], F32, tag="tcol")
                nc.tensor.matmul(gcol_ps[:], grow[:], one_sb[:], start=True, stop=True)
                gcol_sb = chain.tile([B, 1], F32, tag="gcol_sb")
                nc.vector.tensor_copy(gcol_sb[:], gcol_ps[:])
                nc.tensor.matmul(v_next[:], gcol_sb[:], e_sb[:],
                                 start=False, stop=True)
                w = init_w(v_next)
                dm_ps = bps.tile([R, 1], F32, tag="tcol")
                nc.tensor.matmul(
                    dm_ps[:], blk_sl[:B, 2 * B:2 * B + R], gcol_sb[:],
                    start=True, stop=True,
                )
                nc.vector.tensor_tensor(m_sb[:], m_sb[:], dm_ps[:], add)

    _split_multi_waits(nc)
    return nc


_NC_CACHE = None


def _get_nc():
    global _NC_CACHE
    if _NC_CACHE is None:
        _NC_CACHE = _build_nc()
    return _NC_CACHE


def _factor_U(W):
    """Pivoted Cholesky of W+I in fp64; returns U [N,R] fp32 or None."""
    A = W.astype(np.float64) + np.eye(N)
    diag = np.diagonal(A).copy()
    L = np.zeros((N, R))
    for r in range(R):
        j = int(np.argmax(diag))
        if diag[j] < 1e-10:
            L = L[:, :r]
            break
        ljj = np.sqrt(diag[j])
        L[:, r] = (A[:, j] - L[:, :r] @ L[j, :r]) / ljj
        diag -= L[:, r] ** 2
        diag[j] = 0.0
        np.maximum(diag, 0, out=diag)
    U = np.zeros((N, R))
    U[:, :L.shape[1]] = L
    # spot-check the factorization
    idx = np.linspace(0, N - 1, 64).astype(np.int64)
    res = np.abs(U[idx] @ U.T - A[idx]).max()
    return (U.astype(np.float32), float(res))


def _pack_inputs(U, s0, perm):
    Up = U[perm].astype(np.float32)
    s0p = s0[perm].astype(np.float32)
    Ug = (-2.0 * s0p[:, None] * Up).astype(np.float32)
    blk = np.zeros((128, NB * S), dtype=np.float32)
    for b in range(NB):
        sl = slice(b * B, (b + 1) * B)
        blk[:R, b * S + 0:b * S + B] = Up[sl].T
        blk[:R, b * S + B:b * S + 2 * B] = Ug[sl].T
        blk[:B, b * S + 2 * B:b * S + 2 * B + R] = Ug[sl]
        blk[0, b * S + 3 * B:b * S + 4 * B] = -s0p[sl]
    m0 = (U.T.astype(np.float32) @ s0.astype(np.float32))[:, None].astype(np.float32)
    return {"blk": blk, "ns0p": (-s0p)[None, :].astype(np.float32), "m0": m0}


def _sweep_numpy(W, s, perm):
    """Exact fp32 sequential fallback (used only if W is not Hebbian rank-128)."""
    s = s.astype(np.float32).copy()
    for i in perm:
        act = np.float32(np.dot(W[i].astype(np.float32), s))
        s[i] = np.float32(1.0) if act >= 0 else np.float32(-1.0)
    return s


def kernel(W, state, perm, num_iterations):
    W = np.asarray(W, dtype=np.float32)
    state = np.asarray(state, dtype=np.float32)
    perm_i = np.asarray(perm).astype(np.int64)
    n_it = int(np.asarray(num_iterations))

    s = state.copy()
    if n_it <= 0:
        return s

    U, res = _factor_U(W)
    if res > 1e-4:
        for _ in range(n_it):
            s = _sweep_numpy(W, s, perm_i)
        return s

    nc = _get_nc()
    core_ids = list(range(8))
    for _ in range(n_it):
        ins = _pack_inputs(U, s, perm_i)
        r = run_bass_kernel_spmd(nc, [dict(ins) for _ in core_ids], core_ids)
        G = r.results[0]["gout"].reshape(-1)
        flip = perm_i[G > 0.5]
        s[flip] = -s[flip]
    return s
